# revision 12
# baseline (speedup 1.0000x reference)
"""Causal MHA forward on 8 NeuronCores (Trainium2, Bass/Tile).

Sharding: batch (4) x head-half (2) -> 8 cores. Each core computes, for its
batch b and its 8 heads: QKV column-sliced projections (bf16), causal
attention in transposed-score layout (S^T[k, q]), and a partial dense
projection against the matching 512-row slice of dense_w. The host sums the
two partial dense outputs per batch and adds dense_b + wv_b @ dense_w
(valid because softmax rows sum to 1).

Key layout tricks:
- Scores/PV computed per head-pair p (heads 2p, 2p+1). Head 2p's PV lhsT is
  [V_A | ones] -> psum rows 0:64 = O_A, row 64 = rowsum_A. Head 2p+1's lhsT
  is [ones | zeros*63 | V_B] -> psum row 0 = rowsum_B, rows 64:128 = O_B, so
  both heads' outputs land at their final partition ranges with no shift.
- Softmax normalization: reciprocals of the two rowsums -> one bf16 tile
  (rows 0 and 64), two 1-row broadcast matmuls fill a [128,512] psum with
  per-column reciprocals for both heads, then two DVE multiplies.
- Causal masking: diagonal-straddling 128-k-blocks restrict their q range
  to [off:512] (off = 0,128,256,256) and add a bf16 triangle/band mask via
  a small matmul; fully-masked blocks are never computed.
"""
import numpy as np
import ml_dtypes

import concourse.bacc as bacc
import concourse.bass as bass
import concourse.tile as tile
import concourse.mybir as mybir
from concourse.bass_utils import run_bass_kernel_spmd

B, S, D, = 4, 2048, 1024
DC = 512           # per-core d slice (8 heads x 64)
H = 8              # heads per core
DH = 64
N_CORES = 8
F32 = mybir.dt.float32
BF16 = mybir.dt.bfloat16
AF = mybir.ActivationFunctionType
NEG = -1.0e9
SCALE = 1.0 / 32.0  # 1/sqrt(D_MODEL)

# q-range starts for the 4 diagonal-straddling k-blocks of each 512-q chunk
# (last is 256, not 384, to keep matmul moving size >= 256)
OFFS = (0, 128, 256, 256)

_CACHE = {}


def _build():
    nc = bacc.Bacc("TRN2", target_bir_lowering=False, debug=False,
                   num_devices=N_CORES)
    xt = nc.dram_tensor("xt", [D, S], BF16, kind="ExternalInput")
    wq = nc.dram_tensor("wq", [D, DC], BF16, kind="ExternalInput")
    wk = nc.dram_tensor("wk", [D, DC], BF16, kind="ExternalInput")
    wv = nc.dram_tensor("wv", [D, DC], BF16, kind="ExternalInput")
    qb = nc.dram_tensor("qb", [DC], F32, kind="ExternalInput")
    kb = nc.dram_tensor("kb", [DC], F32, kind="ExternalInput")
    wd = nc.dram_tensor("wd", [DC, D], BF16, kind="ExternalInput")
    band = nc.dram_tensor("band", [128, 256], BF16, kind="ExternalInput")
    idm = nc.dram_tensor("idm", [128, 128], BF16, kind="ExternalInput")
    onb = nc.dram_tensor("onb", [128, 128], BF16, kind="ExternalInput")
    out = nc.dram_tensor("out", [S, D], F32, kind="ExternalOutput")

    with tile.TileContext(nc) as tc:
      with nc.allow_low_precision(reason="bf16 storage; all matmul accumulation in fp32 psum"):
        with (
            tc.tile_pool(name="consts", bufs=1) as consts,
            tc.tile_pool(name="ktp", bufs=1) as ktp,
            tc.tile_pool(name="vap", bufs=1) as vap,
            tc.tile_pool(name="otp", bufs=1) as otp,
            tc.tile_pool(name="qtp", bufs=1) as qtp,
            tc.tile_pool(name="xts", bufs=4) as xtsp,
            tc.tile_pool(name="ptp", bufs=2) as ptp,
            tc.tile_pool(name="nrm", bufs=2) as nrm,
            tc.tile_pool(name="psb", bufs=2, space="PSUM") as psb,
            tc.tile_pool(name="psv", bufs=1, space="PSUM") as psv,
            tc.tile_pool(name="psm", bufs=1, space="PSUM") as psm,
            tc.tile_pool(name="wts", bufs=1) as wkvp,
            tc.tile_pool(name="outp", bufs=3) as outp,
        ):
            band_sb = consts.tile([128, 256], BF16)
            nc.gpsimd.dma_start(out=band_sb, in_=band[:, :])
            id_sb = consts.tile([128, 128], BF16)
            nc.gpsimd.dma_start(out=id_sb, in_=idm[:, :])
            onb_sb = consts.tile([128, 128], BF16)
            nc.gpsimd.dma_start(out=onb_sb, in_=onb[:, :])
            qb_sb = consts.tile([128, 4], F32)
            nc.gpsimd.dma_start(out=qb_sb, in_=qb.ap().rearrange("(c p) -> p c", p=128))
            kb_sb = consts.tile([128, 4], F32)
            nc.gpsimd.dma_start(out=kb_sb, in_=kb.ap().rearrange("(c p) -> p c", p=128))

            kt = ktp.tile([128, 4, S], BF16)       # K^T, pair p rows = d 128p..
            qt = qtp.tile([128, 4, S], BF16)       # Q^T, full sequence
            # V per s-block & head pair: [V_A(64) | onesA | onesB | 0*63 | V_B(64)]
            va = vap.tile([128, 16, 4, 193], BF16)
            ot = otp.tile([128, 4, S], BF16)       # O^T accumulated
            nc.vector.memset(va[:, :, :, 64:66], 1.0)
            nc.vector.memset(va[:, :, :, 66:129], 0.0)

            # persistent normalization rhs: rows 0 (1/rowsum_B) and 64
            # (1/rowsum_A) are rewritten per head-pair; rows 1:64 stay zero so
            # a single K=65 broadcast matmul can read the whole tile
            rr2 = nrm.tile([65, 512], BF16, tag="rrP")
            nc.vector.memset(rr2, 0.0)

            wk_sb = wkvp.tile([128, 8, DC], BF16)
            wq_sb = wkvp.tile([128, 8, DC], BF16)
            wv_sb = wkvp.tile([128, 8, DC], BF16)
            wd_sb = wkvp.tile([128, 4, D], BF16)
            wkv = wk.ap().rearrange("(c p) d -> p c d", p=128)
            wqv = wq.ap().rearrange("(c p) d -> p c d", p=128)
            wvv = wv.ap().rearrange("(c p) d -> p c d", p=128)
            wdv = wd.ap().rearrange("(c p) d -> p c d", p=128)
            # sliced weight loads so the first matmuls start early; on the
            # idle Pool queue so they never block the ACT sequencer
            for i in range(8):
                nc.gpsimd.dma_start(out=wk_sb[:, i:i + 1, :], in_=wkv[:, i:i + 1, :])
            for i in range(8):
                nc.gpsimd.dma_start(out=wv_sb[:, i:i + 1, :], in_=wvv[:, i:i + 1, :])
            for i in range(8):
                nc.gpsimd.dma_start(out=wq_sb[:, i:i + 1, :], in_=wqv[:, i:i + 1, :])
            for i in range(4):
                nc.gpsimd.dma_start(out=wd_sb[:, i:i + 1, :], in_=wdv[:, i:i + 1, :])

            xv = xt.ap().rearrange("(i p) s -> p i s", p=128)
            xs = {}
            for sc in range(4):
                x = xtsp.tile([128, 8, 512], BF16, tag="xts", name=f"xts{sc}")
                if sc == 0:
                    for i in range(8):
                        nc.sync.dma_start(out=x[:, i:i + 1, :],
                                          in_=xv[:, i:i + 1, 0:512])
                else:
                    for i in range(4):
                        nc.sync.dma_start(
                            out=x[:, 2 * i:2 * i + 2, :],
                            in_=xv[:, 2 * i:2 * i + 2, 512 * sc:512 * (sc + 1)])
                xs[sc] = x

            # 12 projection groups per chunk (K, V, Q), emitted lazily so
            # they interleave with the ACT-bound attention stream
            def proj_groups(sc):
                xg = xs[sc]
                for p in range(4):
                    def kg(p=p, sc=sc, xg=xg):
                        ps = psm.tile([128, 512], F32, tag="mm", bufs=2, name="kps")
                        for i in range(8):
                            nc.tensor.matmul(ps, wk_sb[:, i, 128 * p:128 * (p + 1)],
                                             xg[:, i, :], start=(i == 0), stop=(i == 7))
                        nc.vector.tensor_scalar_add(
                            out=kt[:, p, 512 * sc:512 * (sc + 1)], in0=ps,
                            scalar1=kb_sb[:, p:p + 1])
                    yield kg
                for sb_ in range(4):
                    def vg(sb_=sb_, sc=sc, xg=xg):
                        ps = psm.tile([128, 512], F32, tag="mm", bufs=2, name="vps")
                        for i in range(8):
                            nc.tensor.matmul(ps, xg[:, i, 128 * sb_:128 * (sb_ + 1)],
                                             wv_sb[:, i, :], start=(i == 0), stop=(i == 7))
                        sblk = 4 * sc + sb_
                        pv2 = ps.rearrange("s (pp two d) -> s pp two d", pp=4, two=2)
                        nc.vector.tensor_copy(out=va[:, sblk, :, 0:64], in_=pv2[:, :, 0, :])
                        nc.vector.tensor_copy(out=va[:, sblk, :, 129:193], in_=pv2[:, :, 1, :])
                    yield vg
                for p in range(4):
                    def qg(p=p, sc=sc, xg=xg):
                        ps = psm.tile([128, 512], F32, tag="mm", bufs=2, name="qps")
                        for i in range(8):
                            nc.tensor.matmul(ps, wq_sb[:, i, 128 * p:128 * (p + 1)],
                                             xg[:, i, :], start=(i == 0), stop=(i == 7))
                        nc.vector.tensor_scalar_add(
                            out=qt[:, p, 512 * sc:512 * (sc + 1)], in0=ps,
                            scalar1=qb_sb[:, p:p + 1])
                    yield qg

            def dense_block(sb_):
                os = outp.tile([128, 1024], F32)
                for n in range(2):
                    ps = psm.tile([128, 512], F32, tag="mm", bufs=2, name="dps")
                    for p in range(4):
                        nc.tensor.matmul(ps, ot[:, p, 128 * sb_:128 * (sb_ + 1)],
                                         wd_sb[:, p, 512 * n:512 * (n + 1)],
                                         start=(p == 0), stop=(p == 3))
                    nc.vector.tensor_copy(out=os[:, 512 * n:512 * (n + 1)], in_=ps)
                nc.sync.dma_start(out=out[128 * sb_:128 * (sb_ + 1), :], in_=os)

            if True:
                for g in proj_groups(0):
                    g()
                for c in range(4):
                    nj = 4 * c + 4
                    nxt = list(proj_groups(c + 1)) if c < 3 else []
                    # diagonal-straddling blocks first: the jj=0 (full-width)
                    # matmul must open every psum accumulation column group
                    order = list(range(4 * c, 4 * c + 4)) + list(range(4 * c))
                    for p in range(4):
                        for g in nxt[3 * p:3 * p + 3]:
                            g()
                        if c > 0:
                            dense_block(4 * (c - 1) + p)
                        pvA = psv.tile([65, 512], F32, tag="pvA", bufs=1, name="pvA")
                        pvB = psv.tile([128, 512], F32, tag="pvB", bufs=1, name="pvB")
                        for idx, j in enumerate(order):
                            jj = j - 4 * c
                            diag = jj >= 0
                            off = OFFS[jj] if diag else 0
                            first, last = idx == 0, idx == nj - 1
                            sc_ps = psb.tile([128, 1024], F32)
                            nc.tensor.matmul(sc_ps[:, off:512],
                                             kt[0:64, p, 128 * j:128 * (j + 1)],
                                             qt[0:64, p, 512 * c + off:512 * (c + 1)],
                                             start=True, stop=not diag)
                            nc.tensor.matmul(sc_ps[:, 512 + off:1024],
                                             kt[64:128, p, 128 * j:128 * (j + 1)],
                                             qt[64:128, p, 512 * c + off:512 * (c + 1)],
                                             start=True, stop=not diag)
                            if diag:
                                if jj < 3:
                                    rh, tp0, tpw = band_sb[:, 128:256], 128 * jj, 128
                                else:
                                    rh, tp0, tpw = band_sb[:, 0:256], 256, 256
                                nc.tensor.matmul(sc_ps[:, tp0:tp0 + tpw], id_sb, rh,
                                                 start=False, stop=True)
                                nc.tensor.matmul(sc_ps[:, 512 + tp0:512 + tp0 + tpw],
                                                 id_sb, rh, start=False, stop=True)
                            pt = ptp.tile([128, 1024], BF16)
                            if off:
                                sc3 = sc_ps.rearrange("p (h q) -> p h q", h=2)[:, :, off:512]
                                pt3 = pt.rearrange("p (h q) -> p h q", h=2)[:, :, off:512]
                                nc.scalar.activation(out=pt3, in_=sc3, func=AF.Exp,
                                                     scale=SCALE)
                            else:
                                nc.scalar.activation(out=pt, in_=sc_ps, func=AF.Exp,
                                                     scale=SCALE)
                            nc.tensor.matmul(pvA[:, off:512], va[:, j, p, 0:65],
                                             pt[:, off:512], start=first, stop=last)
                            nc.tensor.matmul(pvB[:, off:512], va[:, j, p, 65:193],
                                             pt[:, 512 + off:1024], start=first, stop=last)
                        # normalization: rowsum_A at pvA[64], rowsum_B at pvB[0].
                        # Copy both psums to SBUF first so the banks free fast
                        # (next head-pair's PV matmuls reuse them), then work
                        # off the copies.
                        pvAc = nrm.tile([65, 512], BF16, tag="pvAc")
                        pvBc = nrm.tile([128, 512], BF16, tag="pvBc")
                        nc.vector.tensor_copy(out=pvAc, in_=pvA)
                        nc.vector.tensor_copy(out=pvBc, in_=pvB)
                        nc.vector.reciprocal(out=rr2[64:65, :], in_=pvAc[64:65, :])
                        nc.vector.reciprocal(out=rr2[0:1, :], in_=pvBc[0:1, :])
                        bc = psm.tile([128, 512], F32, tag="mm", bufs=2, name="bc")
                        nc.tensor.matmul(bc, onb_sb[0:65, :], rr2[0:65, :],
                                         start=True, stop=True)
                        nc.vector.tensor_mul(out=ot[0:64, p, 512 * c:512 * (c + 1)],
                                             in0=pvAc[0:64, :], in1=bc[0:64, :])
                        nc.vector.tensor_mul(out=ot[64:128, p, 512 * c:512 * (c + 1)],
                                             in0=pvBc[64:128, :], in1=bc[64:128, :])
                # dense for the last chunk's 4 s-blocks
                for sb_ in range(12, 16):
                    dense_block(sb_)
    nc.compile()
    return nc


def get_nc():
    if "nc" not in _CACHE:
        _CACHE["nc"] = _build()
    return _CACHE["nc"]


def kernel(x, mask, wq_w, wq_b, wk_w, wk_b, wv_w, wv_b, dense_w, dense_b,
           _trace=False):
    bf = ml_dtypes.bfloat16
    x = np.asarray(x, dtype=np.float32)
    wq_w = np.asarray(wq_w, np.float32); wq_b = np.asarray(wq_b, np.float32)
    wk_w = np.asarray(wk_w, np.float32); wk_b = np.asarray(wk_b, np.float32)
    wv_w = np.asarray(wv_w, np.float32); wv_b = np.asarray(wv_b, np.float32)
    dense_w = np.asarray(dense_w, np.float32)
    dense_b = np.asarray(dense_b, np.float32)

    # causal masks, bf16: cols 0:128 = all -1e9; cols 128:256 = triangle
    # T[k, qq] = -1e9 where qq < k
    band = np.zeros((128, 256), np.float32)
    band[:, 0:128] = NEG
    k_idx = np.arange(128)[:, None]
    q_idx = np.arange(128)[None, :]
    band[:, 128:256] = np.where(q_idx < k_idx, NEG, 0.0)
    ident = np.eye(128, dtype=np.float32)
    onb = np.zeros((128, 128), np.float32)
    onb[64, 0:64] = 1.0   # broadcasts rowsum_A recip (at partition 64) to rows 0:64
    onb[0, 64:128] = 1.0  # broadcasts rowsum_B recip (at partition 0) to rows 64:128

    in_maps = []
    for core in range(N_CORES):
        b, hh = divmod(core, 2)
        sl = slice(DC * hh, DC * (hh + 1))
        in_maps.append({
            "xt": np.ascontiguousarray(x[b].T).astype(bf),
            "wq": np.ascontiguousarray(wq_w[:, sl]).astype(bf),
            "wk": np.ascontiguousarray(wk_w[:, sl]).astype(bf),
            "wv": np.ascontiguousarray(wv_w[:, sl]).astype(bf),
            "qb": np.ascontiguousarray(wq_b[sl]),
            "kb": np.ascontiguousarray(wk_b[sl]),
            "wd": np.ascontiguousarray(dense_w[sl, :]).astype(bf),
            "band": band.astype(bf), "idm": ident.astype(bf),
            "onb": onb.astype(bf),
        })
    nc = get_nc()
    res = run_bass_kernel_spmd(nc, in_maps, core_ids=list(range(N_CORES)),
                               trace=_trace)
    const = dense_b + wv_b @ dense_w  # bias terms deferred to host
    outs = np.empty((B, S, D), np.float32)
    for b in range(B):
        outs[b] = res.results[2 * b]["out"] + res.results[2 * b + 1]["out"] + const
    if _trace:
        kernel.last_result = res
    return outs


# revision 17
# speedup vs baseline: 1.0681x; 1.0681x over previous
"""Causal MHA forward on 8 NeuronCores (Trainium2, Bass/Tile).

Sharding: batch (4) x head-half (2) -> 8 cores. Each core computes, for its
batch b and its 8 heads: QKV column-sliced projections (bf16), causal
attention in transposed-score layout (S^T[k, q]), and a partial dense
projection against the matching 512-row slice of dense_w. The host sums the
two partial dense outputs per batch and adds dense_b + wv_b @ dense_w
(valid because softmax rows sum to 1).

Key layout tricks:
- Scores/PV computed per head-pair p (heads 2p, 2p+1). Head 2p's PV lhsT is
  [V_A | ones] -> psum rows 0:64 = O_A, row 64 = rowsum_A. Head 2p+1's lhsT
  is [ones | zeros*63 | V_B] -> psum row 0 = rowsum_B, rows 64:128 = O_B, so
  both heads' outputs land at their final partition ranges with no shift.
- Softmax normalization: reciprocals of the two rowsums -> one bf16 tile
  (rows 0 and 64), two 1-row broadcast matmuls fill a [128,512] psum with
  per-column reciprocals for both heads, then two DVE multiplies.
- Causal masking: diagonal-straddling 128-k-blocks restrict their q range
  to [off:512] (off = 0,128,256,256) and add a bf16 triangle/band mask via
  a small matmul; fully-masked blocks are never computed.
"""
import numpy as np
import ml_dtypes

import concourse.bacc as bacc
import concourse.bass as bass
import concourse.tile as tile
import concourse.mybir as mybir
from concourse.bass_utils import run_bass_kernel_spmd

B, S, D, = 4, 2048, 1024
DC = 512           # per-core d slice (8 heads x 64)
H = 8              # heads per core
DH = 64
N_CORES = 8
F32 = mybir.dt.float32
BF16 = mybir.dt.bfloat16
AF = mybir.ActivationFunctionType
NEG = -1.0e9
SCALE = 1.0 / 32.0  # 1/sqrt(D_MODEL)

# q-range starts for the 4 diagonal-straddling k-blocks of each 512-q chunk
# (last is 256, not 384, to keep matmul moving size >= 256)
OFFS = (0, 128, 256, 256)

_CACHE = {}


def _build():
    nc = bacc.Bacc("TRN2", target_bir_lowering=False, debug=False,
                   num_devices=N_CORES)
    xt = nc.dram_tensor("xt", [D, S], BF16, kind="ExternalInput")
    wq = nc.dram_tensor("wq", [D, DC], BF16, kind="ExternalInput")
    wk = nc.dram_tensor("wk", [D, DC], BF16, kind="ExternalInput")
    wv = nc.dram_tensor("wv", [D, DC], BF16, kind="ExternalInput")
    qb = nc.dram_tensor("qb", [DC], F32, kind="ExternalInput")
    kb = nc.dram_tensor("kb", [DC], F32, kind="ExternalInput")
    wd = nc.dram_tensor("wd", [DC, D], BF16, kind="ExternalInput")
    band = nc.dram_tensor("band", [128, 256], BF16, kind="ExternalInput")
    idm = nc.dram_tensor("idm", [128, 128], BF16, kind="ExternalInput")
    onb = nc.dram_tensor("onb", [128, 128], BF16, kind="ExternalInput")
    out = nc.dram_tensor("out", [S, D], F32, kind="ExternalOutput")

    with tile.TileContext(nc) as tc:
      with nc.allow_low_precision(reason="bf16 storage; all matmul accumulation in fp32 psum"):
        with (
            tc.tile_pool(name="consts", bufs=1) as consts,
            tc.tile_pool(name="ktp", bufs=1) as ktp,
            tc.tile_pool(name="vap", bufs=1) as vap,
            tc.tile_pool(name="otp", bufs=1) as otp,
            tc.tile_pool(name="qtp", bufs=1) as qtp,
            tc.tile_pool(name="xts", bufs=4) as xtsp,
            tc.tile_pool(name="ptp", bufs=2) as ptp,
            tc.tile_pool(name="nrm", bufs=2) as nrm,
            tc.tile_pool(name="psb", bufs=2, space="PSUM") as psb,
            tc.tile_pool(name="psv", bufs=1, space="PSUM") as psv,
            tc.tile_pool(name="psm", bufs=1, space="PSUM") as psm,
            tc.tile_pool(name="wts", bufs=1) as wkvp,
            tc.tile_pool(name="outp", bufs=3) as outp,
        ):
            band_sb = consts.tile([128, 256], BF16)
            nc.gpsimd.dma_start(out=band_sb, in_=band[:, :])
            id_sb = consts.tile([128, 128], BF16)
            nc.gpsimd.dma_start(out=id_sb, in_=idm[:, :])
            onb_sb = consts.tile([128, 128], BF16)
            nc.gpsimd.dma_start(out=onb_sb, in_=onb[:, :])
            qb_sb = consts.tile([128, 4], F32)
            nc.gpsimd.dma_start(out=qb_sb, in_=qb.ap().rearrange("(c p) -> p c", p=128))
            kb_sb = consts.tile([128, 4], F32)
            nc.gpsimd.dma_start(out=kb_sb, in_=kb.ap().rearrange("(c p) -> p c", p=128))

            kt = ktp.tile([128, 4, S], BF16)       # K^T, pair p rows = d 128p..
            qt = qtp.tile([128, 4, S], BF16)       # Q^T, full sequence
            # V per s-block & head pair: [V_A(64) | onesA | onesB | 0*63 | V_B(64)]
            va = vap.tile([128, 16, 4, 193], BF16)
            ot = otp.tile([128, 4, S], BF16)       # O^T accumulated
            nc.vector.memset(va[:, :, :, 64:66], 1.0)
            nc.vector.memset(va[:, :, :, 66:129], 0.0)

            # persistent normalization rhs: rows 0 (1/rowsum_B) and 64
            # (1/rowsum_A) are rewritten per head-pair; rows 1:64 stay zero so
            # a single K=65 broadcast matmul can read the whole tile
            rr2 = nrm.tile([65, 512], BF16, tag="rrP")
            nc.vector.memset(rr2, 0.0)

            wk_sb = wkvp.tile([128, 8, DC], BF16)
            wq_sb = wkvp.tile([128, 8, DC], BF16)
            wv_sb = wkvp.tile([128, 8, DC], BF16)
            wd_sb = wkvp.tile([128, 4, D], BF16)
            wkv = wk.ap().rearrange("(c p) d -> p c d", p=128)
            wqv = wq.ap().rearrange("(c p) d -> p c d", p=128)
            wvv = wv.ap().rearrange("(c p) d -> p c d", p=128)
            wdv = wd.ap().rearrange("(c p) d -> p c d", p=128)
            # sliced weight loads so the first matmuls start early; wk/wq on
            # the scalar queue (fast startup), wv/wd on the idle Pool queue
            for i in range(8):
                nc.scalar.dma_start(out=wk_sb[:, i:i + 1, :], in_=wkv[:, i:i + 1, :])
            for i in range(8):
                nc.gpsimd.dma_start(out=wv_sb[:, i:i + 1, :], in_=wvv[:, i:i + 1, :])
            for i in range(8):
                nc.scalar.dma_start(out=wq_sb[:, i:i + 1, :], in_=wqv[:, i:i + 1, :])
            for i in range(4):
                nc.gpsimd.dma_start(out=wd_sb[:, i:i + 1, :], in_=wdv[:, i:i + 1, :])

            xv = xt.ap().rearrange("(i p) s -> p i s", p=128)
            xs = {}
            for sc in range(4):
                x = xtsp.tile([128, 8, 512], BF16, tag="xts", name=f"xts{sc}")
                if sc == 0:
                    for i in range(8):
                        nc.sync.dma_start(out=x[:, i:i + 1, :],
                                          in_=xv[:, i:i + 1, 0:512])
                else:
                    for i in range(4):
                        nc.sync.dma_start(
                            out=x[:, 2 * i:2 * i + 2, :],
                            in_=xv[:, 2 * i:2 * i + 2, 512 * sc:512 * (sc + 1)])
                xs[sc] = x

            # filler thunks: one matmul each, interleaved between attention
            # j-iterations so the (ACT-bound) attention stream never leaves
            # PE idle. A group's psum tile is allocated by its first thunk
            # and finalized (DVE drain) by its last.
            def group_thunks(make_mm, n_acc, finalize, name):
                box = []
                for i in range(n_acc):
                    def t(i=i, box=box):
                        if i == 0:
                            box.append(psm.tile([128, 512], F32, tag="mm",
                                                bufs=2, name=name))
                        make_mm(box[0], i)
                        if i == n_acc - 1:
                            finalize(box[0])
                    yield t

            def proj_thunks(sc):
                xg = xs[sc]
                th = []
                for p in range(4):
                    th += list(group_thunks(
                        lambda ps, i, p=p: nc.tensor.matmul(
                            ps, wk_sb[:, i, 128 * p:128 * (p + 1)], xg[:, i, :],
                            start=(i == 0), stop=(i == 7)),
                        8,
                        lambda ps, p=p: nc.vector.tensor_scalar_add(
                            out=kt[:, p, 512 * sc:512 * (sc + 1)], in0=ps,
                            scalar1=kb_sb[:, p:p + 1]),
                        "kps"))
                for sb_ in range(4):
                    def vfin(ps, sb_=sb_):
                        sblk = 4 * sc + sb_
                        pv2 = ps.rearrange("s (pp two d) -> s pp two d", pp=4, two=2)
                        nc.vector.tensor_copy(out=va[:, sblk, :, 0:64],
                                              in_=pv2[:, :, 0, :])
                        nc.vector.tensor_copy(out=va[:, sblk, :, 129:193],
                                              in_=pv2[:, :, 1, :])
                    th += list(group_thunks(
                        lambda ps, i, sb_=sb_: nc.tensor.matmul(
                            ps, xg[:, i, 128 * sb_:128 * (sb_ + 1)], wv_sb[:, i, :],
                            start=(i == 0), stop=(i == 7)),
                        8, vfin, "vps"))
                for p in range(4):
                    th += list(group_thunks(
                        lambda ps, i, p=p: nc.tensor.matmul(
                            ps, wq_sb[:, i, 128 * p:128 * (p + 1)], xg[:, i, :],
                            start=(i == 0), stop=(i == 7)),
                        8,
                        lambda ps, p=p: nc.vector.tensor_scalar_add(
                            out=qt[:, p, 512 * sc:512 * (sc + 1)], in0=ps,
                            scalar1=qb_sb[:, p:p + 1]),
                        "qps"))
                return th

            def dense_thunks(cd):
                th = []
                for sb_ in range(4 * cd, 4 * cd + 4):
                    os_box = []
                    for n in range(2):
                        def mk(ps, i, n=n, sb_=sb_, os_box=os_box):
                            if n == 0 and i == 0:
                                os_box.append(outp.tile([128, 1024], F32, name="os"))
                            nc.tensor.matmul(ps, ot[:, i, 128 * sb_:128 * (sb_ + 1)],
                                             wd_sb[:, i, 512 * n:512 * (n + 1)],
                                             start=(i == 0), stop=(i == 3))
                        def dfin(ps, n=n, sb_=sb_, os_box=os_box):
                            nc.vector.tensor_copy(
                                out=os_box[0][:, 512 * n:512 * (n + 1)], in_=ps)
                            if n == 1:
                                nc.sync.dma_start(
                                    out=out[128 * sb_:128 * (sb_ + 1), :],
                                    in_=os_box[0])
                        th += list(group_thunks(mk, 4, dfin, "dps"))
                return th

            if True:
                for t in proj_thunks(0):
                    t()
                for c in range(4):
                    nj = 4 * c + 4
                    filler = []
                    if c < 3:
                        filler += proj_thunks(c + 1)
                    if c > 0:
                        filler += dense_thunks(c - 1)
                    nf = len(filler)
                    # diagonal-straddling blocks first: the jj=0 (full-width)
                    # matmul must open every psum accumulation column group
                    order = list(range(4 * c, 4 * c + 4)) + list(range(4 * c))
                    for p in range(4):
                        fil = filler[nf * p // 4: nf * (p + 1) // 4]
                        pvA = psv.tile([65, 512], F32, tag="pvA", bufs=1, name="pvA")
                        pvB = psv.tile([128, 512], F32, tag="pvB", bufs=1, name="pvB")
                        for idx, j in enumerate(order):
                            jj = j - 4 * c
                            diag = jj >= 0
                            off = OFFS[jj] if diag else 0
                            first, last = idx == 0, idx == nj - 1
                            sc_ps = psb.tile([128, 1024], F32)
                            nc.tensor.matmul(sc_ps[:, off:512],
                                             kt[0:64, p, 128 * j:128 * (j + 1)],
                                             qt[0:64, p, 512 * c + off:512 * (c + 1)],
                                             start=True, stop=not diag)
                            nc.tensor.matmul(sc_ps[:, 512 + off:1024],
                                             kt[64:128, p, 128 * j:128 * (j + 1)],
                                             qt[64:128, p, 512 * c + off:512 * (c + 1)],
                                             start=True, stop=not diag)
                            if diag:
                                if jj < 3:
                                    rh, tp0, tpw = band_sb[:, 128:256], 128 * jj, 128
                                else:
                                    rh, tp0, tpw = band_sb[:, 0:256], 256, 256
                                nc.tensor.matmul(sc_ps[:, tp0:tp0 + tpw], id_sb, rh,
                                                 start=False, stop=True)
                                nc.tensor.matmul(sc_ps[:, 512 + tp0:512 + tp0 + tpw],
                                                 id_sb, rh, start=False, stop=True)
                            pt = ptp.tile([128, 1024], BF16)
                            if off:
                                sc3 = sc_ps.rearrange("p (h q) -> p h q", h=2)[:, :, off:512]
                                pt3 = pt.rearrange("p (h q) -> p h q", h=2)[:, :, off:512]
                                nc.scalar.activation(out=pt3, in_=sc3, func=AF.Exp,
                                                     scale=SCALE)
                            else:
                                nc.scalar.activation(out=pt, in_=sc_ps, func=AF.Exp,
                                                     scale=SCALE)
                            nc.tensor.matmul(pvA[:, off:512], va[:, j, p, 0:65],
                                             pt[:, off:512], start=first, stop=last)
                            nc.tensor.matmul(pvB[:, off:512], va[:, j, p, 65:193],
                                             pt[:, 512 + off:1024], start=first, stop=last)
                            for t in fil[len(fil) * idx // nj:
                                         len(fil) * (idx + 1) // nj]:
                                t()
                        # normalization: rowsum_A at pvA[64], rowsum_B at pvB[0].
                        # Copy both psums to SBUF first so the banks free fast
                        # (next head-pair's PV matmuls reuse them), then work
                        # off the copies.
                        pvAc = nrm.tile([65, 512], BF16, tag="pvAc")
                        pvBc = nrm.tile([128, 512], BF16, tag="pvBc")
                        nc.vector.tensor_copy(out=pvAc, in_=pvA)
                        nc.vector.tensor_copy(out=pvBc, in_=pvB)
                        nc.vector.reciprocal(out=rr2[64:65, :], in_=pvAc[64:65, :])
                        nc.vector.reciprocal(out=rr2[0:1, :], in_=pvBc[0:1, :])
                        bc = psm.tile([128, 512], F32, tag="mm", bufs=2, name="bc")
                        nc.tensor.matmul(bc, onb_sb[0:65, :], rr2[0:65, :],
                                         start=True, stop=True)
                        nc.vector.tensor_mul(out=ot[0:64, p, 512 * c:512 * (c + 1)],
                                             in0=pvAc[0:64, :], in1=bc[0:64, :])
                        nc.vector.tensor_mul(out=ot[64:128, p, 512 * c:512 * (c + 1)],
                                             in0=pvBc[64:128, :], in1=bc[64:128, :])
                # dense for the last chunk's 4 s-blocks
                for t in dense_thunks(3):
                    t()
    nc.compile()
    return nc


def get_nc():
    if "nc" not in _CACHE:
        _CACHE["nc"] = _build()
    return _CACHE["nc"]


def kernel(x, mask, wq_w, wq_b, wk_w, wk_b, wv_w, wv_b, dense_w, dense_b,
           _trace=False):
    bf = ml_dtypes.bfloat16
    x = np.asarray(x, dtype=np.float32)
    wq_w = np.asarray(wq_w, np.float32); wq_b = np.asarray(wq_b, np.float32)
    wk_w = np.asarray(wk_w, np.float32); wk_b = np.asarray(wk_b, np.float32)
    wv_w = np.asarray(wv_w, np.float32); wv_b = np.asarray(wv_b, np.float32)
    dense_w = np.asarray(dense_w, np.float32)
    dense_b = np.asarray(dense_b, np.float32)

    # causal masks, bf16: cols 0:128 = all -1e9; cols 128:256 = triangle
    # T[k, qq] = -1e9 where qq < k
    band = np.zeros((128, 256), np.float32)
    band[:, 0:128] = NEG
    k_idx = np.arange(128)[:, None]
    q_idx = np.arange(128)[None, :]
    band[:, 128:256] = np.where(q_idx < k_idx, NEG, 0.0)
    ident = np.eye(128, dtype=np.float32)
    onb = np.zeros((128, 128), np.float32)
    onb[64, 0:64] = 1.0   # broadcasts rowsum_A recip (at partition 64) to rows 0:64
    onb[0, 64:128] = 1.0  # broadcasts rowsum_B recip (at partition 0) to rows 64:128

    in_maps = []
    for core in range(N_CORES):
        b, hh = divmod(core, 2)
        sl = slice(DC * hh, DC * (hh + 1))
        in_maps.append({
            "xt": np.ascontiguousarray(x[b].T).astype(bf),
            "wq": np.ascontiguousarray(wq_w[:, sl]).astype(bf),
            "wk": np.ascontiguousarray(wk_w[:, sl]).astype(bf),
            "wv": np.ascontiguousarray(wv_w[:, sl]).astype(bf),
            "qb": np.ascontiguousarray(wq_b[sl]),
            "kb": np.ascontiguousarray(wk_b[sl]),
            "wd": np.ascontiguousarray(dense_w[sl, :]).astype(bf),
            "band": band.astype(bf), "idm": ident.astype(bf),
            "onb": onb.astype(bf),
        })
    nc = get_nc()
    res = run_bass_kernel_spmd(nc, in_maps, core_ids=list(range(N_CORES)),
                               trace=_trace)
    const = dense_b + wv_b @ dense_w  # bias terms deferred to host
    outs = np.empty((B, S, D), np.float32)
    for b in range(B):
        outs[b] = res.results[2 * b]["out"] + res.results[2 * b + 1]["out"] + const
    if _trace:
        kernel.last_result = res
    return outs
